# revision 25
# baseline (speedup 1.0000x reference)
"""BioGNN message-passing kernel for 8 trn2 NeuronCores.

Strategy:
  - Shard by DESTINATION node range: core c owns nodes [c*125k, (c+1)*125k).
    Each edge is routed (host-side layout) to the core owning its dst, so no
    all-reduce is needed; the host concatenates per-core output slices.
  - Host does LAYOUT ONLY: per owned node, incoming edges are padded into
    dense ELL slabs binned by in-degree class; each slot carries bf16(x[src])
    (and bf16(k) when k is not all-ones). Nodes are ordered bin-major
    ((Ka,Ki) lexicographic) so per-class slab regions reduce into contiguous
    sum slices. Outputs are un-permuted on the host.
  - Device does ALL arithmetic: ScalarE (+ Pool for a slice share) squares
    the bf16 slots in place per window slice; VectorE reduces each K-group
    via a bf16 pairwise-halving tree (2x DVE mode) finishing in f32. Tree
    levels are SHARED across all same-K chunks (slab is laid out grouped by
    K), only the final level is per-destination-slice. Then the elementwise
    tail (num/den ratio, decay/growth terms) spread across engines.
  - num/den masks are folded into data/layout: asum rows for no-activator
    nodes are memset to 1 (isolated nodes to 0; bin (0,0) sorts first);
    nodes with act-degree 0 that were promoted into a padded class carry one
    pad slot of 1.0 so their reduced sum is exactly the mask value.
"""

import contextlib

import ml_dtypes
import numpy as np

import concourse.bacc as bacc
import concourse.mybir as mybir
import concourse.tile as tile
from concourse.bass_utils import run_bass_kernel_spmd

N_NODES = 1_000_000
N_CORES = 8
NPC = N_NODES // N_CORES
P = 128

WINDOW = 8192     # slab window width per partition, in bf16 words
TREE_MIN = 512    # groups narrower than this use per-sub tensor_reduce
RARE = 16384      # consolidate (Ka,Ki) pairs with fewer nodes than this
KCAP = 16         # rare pairs are promoted to at least this class
SQ_POOL_FRAC = 0.0  # Pool bf16 tensor ops are ~7x slower on HW than modeled
SQ_SLICE = 3072   # max bf16 words per square op (pipelining granularity)
POOL_FIN = False  # width-2 inh finals run on Pool instead of VectorE
ABLATE = frozenset()  # debug: subsets of {"dma","sq","tree","tail"} to skip
DEFER_POOL_FIN = True  # emit Pool finals after the window loop
TAIL_ADD = "a"    # engine for isum+=1: "a" ScalarE, "v" VectorE
OUT_Q = "s"       # queue issuing output DMAs: "a" ScalarE, "s" SP, "p" Pool

F32 = mybir.dt.float32
BF16 = mybir.dt.bfloat16


FINE_CLASSES = False

def _degree_classes(max_deg: int) -> list[int]:
    ks = (
        [4, 6, 8, 10, 12, 14, 16, 20, 24, 32]
        if FINE_CLASSES
        else [4, 6, 8, 12, 16, 24, 32]
    )
    ks = list(ks)
    while ks[-1] < max_deg:
        ks.append(ks[-1] * 2)
    return ks


def _class_of(deg: np.ndarray, ks: list[int]) -> np.ndarray:
    bounds = np.array(ks)
    idx = np.searchsorted(bounds, deg, side="left")
    out = np.zeros_like(deg)
    nz = deg > 0
    out[nz] = bounds[idx[nz]]
    return out


def _pack_bf16_words(arr):
    """[P, n] f32 -> [P, ceil(n/2)] f32 words holding round-to-nearest bf16."""
    a = arr.astype(ml_dtypes.bfloat16)
    if a.shape[1] % 2:
        a = np.concatenate([a, np.zeros((a.shape[0], 1), ml_dtypes.bfloat16)], axis=1)
    u = a.view(np.uint16)
    w = (u[:, 0::2].astype(np.uint32) | (u[:, 1::2].astype(np.uint32) << 16)).view(
        np.float32
    )
    return np.ascontiguousarray(w)


def _tree_steps(K: int):
    """Halving widths (bf16 tt levels) and the final width (f32 finish)."""
    w = K
    steps = []
    while w % 2 == 0 and w > 2:
        w //= 2
        steps.append(w)
    return steps, w


def _make_plan(all_keys, nrows, k1):
    """Group chunks by K class, pack class groups into windows.

    Returns (entries, windows, row_off, act_seg_rows, total_rows):
      entries: flat list of (table, K, g0, t, win, woff) for the packer,
        woff = bf16-word offset of the x region within the window (the k
        region, general path only, sits at woff + t*K).
      windows: list of {used, groups: [{K, woff, width, subs}]} where subs
        are (table, g0, t, soff) with soff the sub's slot offset (in slots
        of K words) within the group.
    """
    row_off = {}
    off = 0
    for key in all_keys:
        row_off[key] = off
        off += nrows[key]
    total_rows = off

    mult = 1 if k1 else 2
    # per-class chunk lists: act as one run of contiguous rows per class,
    # inh per bin
    act_seg_rows = {}
    by_k = {}
    for key in all_keys:
        Ka, Ki = key
        if Ka > 0 and Ka not in act_seg_rows:
            seg_rows = sum(nrows[k] for k in all_keys if k[0] == Ka)
            act_seg_rows[Ka] = (row_off[key], seg_rows)
            by_k.setdefault(Ka, []).append(("a", row_off[key], seg_rows))
        if Ki > 0:
            by_k.setdefault(Ki, []).append(("i", row_off[key], nrows[key]))

    # pack class groups into windows; split oversized groups at chunk
    # boundaries (chunks themselves split to fit WINDOW slots)
    entries = []
    windows = []
    cur = {"used": 0, "groups": []}

    def close_window():
        nonlocal cur
        if cur["groups"]:
            windows.append(cur)
            cur = {"used": 0, "groups": []}

    for K in sorted(by_k, key=lambda k: -sum(c[2] for c in by_k[k]) * k):
        max_slots = WINDOW // (K * mult)
        # split per-table runs into chunks of at most max_slots rows
        chunks = []
        for table, g0, rows in by_k[K]:
            r = 0
            while r < rows:
                t = min(max_slots, rows - r)
                chunks.append((table, g0 + r, t))
                r += t
        ci = 0
        while ci < len(chunks):
            free = WINDOW - cur["used"]
            grp_slots = free // (K * mult)
            if grp_slots < chunks[ci][2] and grp_slots < max_slots:
                close_window()
                continue
            grp = {"K": K, "woff": cur["used"], "subs": []}
            soff = 0
            while ci < len(chunks) and soff + chunks[ci][2] <= grp_slots:
                table, g0, t = chunks[ci]
                grp["subs"].append((table, g0, t, soff))
                entries.append((table, K, g0, t, len(windows),
                                grp["woff"] + soff * K * mult))
                soff += t
                ci += 1
            grp["width"] = soff * K
            cur["used"] += -(-(soff * K * mult) // 64) * 64
            cur["groups"].append(grp)
            if cur["used"] >= WINDOW - 64:
                close_window()
    close_window()
    return entries, windows, row_off, act_seg_rows, total_rows


def _pack(x, k_act, k_inh, nu, decay, growth, act_src, act_dst, inh_src, inh_dst):
    k1 = bool(np.all(k_act == 1.0) and np.all(k_inh == 1.0))
    ndg1 = bool(np.all(nu == 1.0) and np.all(decay == 1.0) and np.all(growth == 1.0))

    def sorted_table(src, dst, k):
        order = np.argsort(dst, kind="stable")
        deg = np.bincount(dst, minlength=N_NODES).astype(np.int64)
        rowptr = np.zeros(N_NODES + 1, np.int64)
        np.cumsum(deg, out=rowptr[1:])
        return src[order], k[order], deg, rowptr

    a_src, a_k, a_deg, a_ptr = sorted_table(act_src, act_dst, k_act)
    i_src, i_k, i_deg, i_ptr = sorted_table(inh_src, inh_dst, k_inh)

    max_deg = int(max(a_deg.max(), i_deg.max()))
    ks = _degree_classes(max_deg)
    nclasses = len(ks) + 1
    klist = [0] + ks

    ca = _class_of(a_deg, ks)
    ci = _class_of(i_deg, ks)

    # consolidate rare (ca, ci) pairs by cascading each into the cheapest
    # neighbour pair (bump one class up) until populous, so the device sees
    # few, large chunks without the padding blowup of a fixed promotion
    # target. Pair (0,0) (isolated nodes) is exempt: its rows must stay
    # identifiable so asum can be zeroed for them.
    karr = np.array([0] + ks)

    def up(c):
        i = int(np.searchsorted(karr, c)) + 1
        return int(karr[min(i, len(karr) - 1)])

    pair_id = ca * 1024 + ci
    uniq_p, cnt_p = np.unique(pair_id, return_counts=True)
    pop = {int(u): int(n) for u, n in zip(uniq_p, cnt_p)}
    remap = {}
    live = dict(pop)
    changed = True
    while changed:
        changed = False
        for pid in sorted(live, key=lambda q: live[q]):
            if live[pid] >= RARE or pid == 0:
                continue
            a, i = pid // 1024, pid % 1024
            cands = []
            ua, ui = up(a), up(i)
            if ua != a:
                cands.append(((ua - a), ua * 1024 + i))
            if ui != i:
                cands.append(((ui - i), a * 1024 + ui))
            if not cands:
                continue
            cands.sort()
            _, tgt = cands[0]
            remap[pid] = tgt
            live[tgt] = live.get(tgt, 0) + live[pid]
            del live[pid]
            changed = True
            break

    def resolve(pid):
        while pid in remap:
            pid = remap[pid]
        return pid

    if remap:
        res = {int(u): resolve(int(u)) for u in uniq_p}
        new_id = np.vectorize(res.get, otypes=[np.int64])(pair_id)
        ca = (new_id // 1024).astype(ca.dtype)
        ci = (new_id % 1024).astype(ci.dtype)

    x32 = x.astype(np.float32)
    xbf = x32.astype(ml_dtypes.bfloat16).astype(np.float32)

    core_bins = []
    for c in range(N_CORES):
        lo, hi = c * NPC, (c + 1) * NPC
        nodes = np.arange(lo, hi)
        binid = np.searchsorted(np.array(klist), ca[lo:hi]) * nclasses + np.searchsorted(
            np.array(klist), ci[lo:hi]
        )
        order = np.argsort(binid, kind="stable")
        nodes_sorted = nodes[order]
        binid_sorted = binid[order]
        uniq, starts = np.unique(binid_sorted, return_index=True)
        ends = np.append(starts[1:], len(binid_sorted))
        bins = {}
        for u, s, e in zip(uniq, starts, ends):
            bins[(klist[u // nclasses], klist[u % nclasses])] = nodes_sorted[s:e]
        core_bins.append(bins)

    all_keys = sorted({k for b in core_bins for k in b.keys()})
    nrows = {}
    for key in all_keys:
        nmax = max(len(b.get(key, ())) for b in core_bins)
        nrows[key] = -(-nmax // P)

    entries, windows, row_off, act_seg_rows, total_rows = _make_plan(
        all_keys, nrows, k1
    )

    r000 = nrows.get((0, 0), 0) if all_keys and all_keys[0] == (0, 0) else 0
    rka0 = sum(nrows[k] for k in all_keys if k[0] == 0)
    iz_runs = [(row_off[k], nrows[k]) for k in all_keys if k[1] == 0]

    def build_slab(L, K, rowptr, deg, srcs, kvals, act_pad1):
        Lc = L.clip(0)
        d = np.where(L >= 0, deg[Lc], 0)
        base = rowptr[Lc]
        cols = np.arange(K)
        idx2 = base[:, None] + cols[None, :]
        valid = cols[None, :] < d[:, None]
        idxc = np.where(valid, idx2, 0)
        sx = np.where(valid, xbf[srcs[idxc]], np.float32(0))
        sk = np.where(valid, kvals[idxc], np.float32(0)) if not k1 else None
        if act_pad1:
            z = (L >= 0) & (d == 0)
            if z.any():
                sx[z, 0] = 1.0
                if sk is not None:
                    sk[z, 0] = 1.0
        return sx, sk

    win_used = [w0["used"] for w0 in windows]
    win_start = [0]
    for u in win_used[:-1]:
        win_start.append(win_start[-1] + u)

    per_core = []
    meta_orders = []
    for c in range(N_CORES):
        bins = core_bins[c]
        ax_parts = {}
        ak_parts = {}
        ix_bin = {}
        ik_bin = {}
        xv_l = []
        ndg_l = {"nu": [], "de": [], "gr": []}
        orders = []
        for key in all_keys:
            Ka, Ki = key
            nr = nrows[key]
            L = np.full(nr * P, -1, np.int64)
            have = bins.get(key)
            if have is not None:
                L[: len(have)] = have
            orders.append((key, L))
            if Ka > 0:
                sx, sk = build_slab(L, Ka, a_ptr, a_deg, a_src, a_k, True)
                ax_parts.setdefault(Ka, []).append(sx.reshape(P, nr * Ka))
                if sk is not None:
                    ak_parts.setdefault(Ka, []).append(sk.reshape(P, nr * Ka))
            if Ki > 0:
                sx, sk = build_slab(L, Ki, i_ptr, i_deg, i_src, i_k, False)
                ix_bin[key] = sx.reshape(P, nr * Ki)
                if sk is not None:
                    ik_bin[key] = sk.reshape(P, nr * Ki)
            valid = L >= 0
            Lc = L.clip(0)

            def pk(v):
                return (
                    np.where(valid, v[Lc], np.float32(0))
                    .astype(np.float32)
                    .reshape(P, nr)
                )

            xv_l.append(pk(x32))
            if not ndg1:
                ndg_l["nu"].append(pk(nu))
                ndg_l["de"].append(pk(decay))
                ndg_l["gr"].append(pk(growth))

        ax_seg = {Ka: np.concatenate(p, axis=1) for Ka, p in ax_parts.items()}
        ak_seg = {Ka: np.concatenate(p, axis=1) for Ka, p in ak_parts.items()}

        slab = np.zeros((P, (win_start[-1] + win_used[-1]) // 2), np.float32)
        for table, K, g0, t, win, woff in entries:
            w = t * K
            base = win_start[win] + woff
            if table == "a":
                seg0, _ = act_seg_rows[K]
                r0 = g0 - seg0
                sx = ax_seg[K][:, r0 * K : (r0 + t) * K]
                sk = ak_seg[K][:, r0 * K : (r0 + t) * K] if not k1 else None
            else:
                key = next(
                    kk for kk in all_keys
                    if kk[1] == K and row_off[kk] <= g0 < row_off[kk] + nrows[kk]
                )
                r0 = g0 - row_off[key]
                sx = ix_bin[key][:, r0 * K : (r0 + t) * K]
                sk = ik_bin[key][:, r0 * K : (r0 + t) * K] if not k1 else None
            slab[:, base // 2 : base // 2 + w // 2] = _pack_bf16_words(sx)
            if sk is not None:
                kb = base + w
                slab[:, kb // 2 : kb // 2 + w // 2] = _pack_bf16_words(sk)

        core = {"slab": slab, "nodevf": np.ascontiguousarray(np.concatenate(xv_l, axis=1))}
        if not ndg1:
            ndg = np.concatenate(
                [np.concatenate(ndg_l[nm], axis=1) for nm in ("nu", "de", "gr")],
                axis=1,
            )
            core["nodevb"] = _pack_bf16_words(ndg)
        per_core.append(core)
        meta_orders.append(orders)

    shapes = {
        "keys": all_keys,
        "nrows": nrows,
        "NR": total_rows,
        "entries": entries,
        "windows": windows,
        "win_used": win_used,
        "win_start": win_start,
        "r000": r000,
        "rka0": rka0,
        "iz_runs": iz_runs,
        "k1": k1,
        "ndg1": ndg1,
    }
    assert per_core[0]["nodevf"].shape[1] == shapes["NR"]
    return per_core, meta_orders, shapes


def _build_nc(shapes, loop_R=None, unroll=1):
    NR = shapes["NR"]
    windows = shapes["windows"]
    win_used = shapes["win_used"]
    win_start = shapes["win_start"]
    r000 = shapes["r000"]
    rka0 = shapes["rka0"]
    iz_runs = shapes["iz_runs"]
    k1 = shapes["k1"]
    ndg1 = shapes["ndg1"]

    nc = bacc.Bacc(None, target_bir_lowering=False)
    WT2 = (win_start[-1] + win_used[-1]) // 2
    sl_d = nc.declare_dram_parameter("slab", [P, WT2], F32, isOutput=False)
    nvf_d = nc.declare_dram_parameter("nodevf", [P, NR], F32, isOutput=False)
    if not ndg1:
        NB = -(-3 * NR // 2)
        nvb_d = nc.declare_dram_parameter("nodevb", [P, NB], F32, isOutput=False)
    out_d = nc.declare_dram_parameter("out", [P, NR], F32, isOutput=True)

    MUL = mybir.AluOpType.mult
    ADD = mybir.AluOpType.add
    SUB = mybir.AluOpType.subtract
    X = mybir.AxisListType.X

    # square-slice engine assignment: greedy fill Pool up to SQ_POOL_FRAC
    total_sq = sum(g["width"] for w0 in windows for g in w0["groups"])
    pool_budget = total_sq * SQ_POOL_FRAC

    with tile.TileContext(nc) as tc:
        with (
            tc.tile_pool(name="slab", bufs=4) as slab_tp,
            tc.tile_pool(name="tree", bufs=3) as tree_tp,
            tc.tile_pool(name="sums", bufs=1) as sums_tp,
            tc.tile_pool(name="node", bufs=1) as node_tp,
        ):
            U = unroll
            if loop_R:
                for cand in (8, 4, 2, 1):
                    if loop_R % cand == 0:
                        U = cand
                        break
            loop_cm = (
                tc.For_i(0, loop_R // U, 1) if loop_R else contextlib.nullcontext()
            )
            with loop_cm:
              for _body_rep in range(U):
                asum = sums_tp.tile([P, NR], F32, tag="asum")
                isum = sums_tp.tile([P, NR], F32, tag="isum")
                if r000 > 0:
                    nc.gpsimd.memset(asum[:, :r000], 0.0)
                if rka0 > r000:
                    nc.gpsimd.memset(asum[:, r000:rka0], 1.0)
                for z0, zn in iz_runs:
                    nc.gpsimd.memset(isum[:, z0 : z0 + zn], 0.0)

                xv = node_tp.tile([P, NR], F32, tag="xv")
                nc.scalar.dma_start(out=xv[:, :], in_=nvf_d[:, :])
                if not ndg1:
                    NB = -(-3 * NR // 2)
                    nvb = node_tp.tile([P, NB], F32, tag="nvb")
                    nc.scalar.dma_start(out=nvb[:, :], in_=nvb_d[:, :])
                    nvb_b = nvb[:, :].bitcast(BF16)
                    iv = {
                        nm: nvb_b[:, j * NR : (j + 1) * NR]
                        for j, nm in enumerate(("nu", "de", "gr"))
                    }

                sq_state = {"a": 0.0, "p": 0.0}

                def square_region(xs, width):
                    # slice into <=SQ_SLICE pieces, balance ACT vs Pool by
                    # weighted time (ACT 0.833 ns/el, Pool 1.98 ns/el)
                    o = 0
                    while o < width:
                        w = min(SQ_SLICE, width - o)
                        sl = xs[:, o : o + w]
                        use_pool = (
                            SQ_POOL_FRAC > 0
                            and sq_state["p"] + w
                            <= (sq_state["a"] + sq_state["p"] + w) * SQ_POOL_FRAC
                        )
                        if use_pool:
                            with nc.allow_low_precision(reason="bf16 squares"):
                                nc.gpsimd.tensor_tensor(out=sl, in0=sl, in1=sl, op=MUL)
                            sq_state["p"] += w
                        else:
                            nc.scalar.square(out=sl, in_=sl)
                            sq_state["a"] += w
                        o += w

                bufs = {"a": asum, "i": isum}
                deferred_fin = []
                for win, w0 in enumerate(windows):
                    used2 = -(-win_used[win] // 2)
                    wt = slab_tp.tile([P, WINDOW // 2], F32, tag="win")
                    ws2 = win_start[win] // 2
                    # split the window DMA at a group boundary near the middle
                    split2 = 0
                    for g in w0["groups"]:
                        if g["woff"] // 2 >= used2 // 2:
                            split2 = g["woff"] // 2
                            break
                    if "dma" not in ABLATE:
                        if 0 < split2 < used2:
                            nc.sync.dma_start(
                                out=wt[:, :split2], in_=sl_d[:, ws2 : ws2 + split2]
                            )
                            nc.sync.dma_start(
                                out=wt[:, split2:used2],
                                in_=sl_d[:, ws2 + split2 : ws2 + used2],
                            )
                        else:
                            nc.sync.dma_start(
                                out=wt[:, :used2], in_=sl_d[:, ws2 : ws2 + used2]
                            )
                    else:
                        nc.vector.memset(wt[:, :64], 1.0)
                    for g in w0["groups"]:
                        K = g["K"]
                        width = g["width"]
                        if width == 0:
                            continue
                        o2 = g["woff"] // 2
                        xs = wt[:, o2 : o2 + -(-width // 2)].bitcast(BF16)[:, :width]
                        if k1:
                            if "sq" not in ABLATE:
                                square_region(xs, width)
                        else:
                            # per-sub k regions sit interleaved: sub x at
                            # soff*K*2, k at soff*K*2 + t*K (see _make_plan)
                            with nc.allow_low_precision(reason="bf16 edge gains"):
                                for table, g0, t, soff in g["subs"]:
                                    w = t * K
                                    xo2 = (g["woff"] + soff * K * 2) // 2
                                    xsk = wt[:, xo2 : xo2 + -(-w // 2)].bitcast(
                                        BF16
                                    )[:, :w]
                                    kS = wt[
                                        :, xo2 + -(-w // 2) : xo2 + 2 * -(-w // 2)
                                    ].bitcast(BF16)[:, :w]
                                    square_region(xsk, w)
                                    nc.vector.tensor_tensor(
                                        out=xsk, in0=xsk, in1=kS, op=MUL
                                    )
                        steps, fin = _tree_steps(K)
                        if "tree" in ABLATE:
                            continue
                        if width < TREE_MIN:
                            for table, g0, t, soff in g["subs"]:
                                if k1:
                                    sub = xs[:, soff * K : (soff + t) * K]
                                else:
                                    xo2 = (g["woff"] + soff * K * 2) // 2
                                    sub = wt[:, xo2 : xo2 + -(-(t * K) // 2)].bitcast(
                                        BF16
                                    )[:, : t * K]
                                nc.vector.tensor_reduce(
                                    out=bufs[table][:, g0 : g0 + t],
                                    in_=sub.rearrange("p (t k) -> p t k", k=K),
                                    axis=X,
                                    op=ADD,
                                )
                            continue
                        # shared halving levels over the whole group
                        if k1:
                            cur = xs.rearrange("p (t k) -> p t k", k=K)
                        else:
                            # general path: x slots per sub are strided by
                            # the k region; fall back to per-sub trees
                            cur = None
                        curw = K
                        if cur is not None:
                            tr = tree_tp.tile([P, WINDOW // 2], BF16, tag="tr")
                            tslots = width // K
                            with nc.allow_low_precision(reason="bf16 tree"):
                                for hw in steps:
                                    dst = tr[:, : tslots * hw].rearrange(
                                        "p (t k) -> p t k", k=hw
                                    )
                                    nc.vector.tensor_tensor(
                                        out=dst,
                                        in0=cur[:, :, :hw],
                                        in1=cur[:, :, hw:curw],
                                        op=ADD,
                                    )
                                    cur = dst
                                    curw = hw
                            f3t = None
                            for table, g0, t, soff in g["subs"]:
                                sub = cur[:, soff : soff + t, :]
                                sumsl = bufs[table][:, g0 : g0 + t]
                                if fin == 2:
                                    if POOL_FIN and table == "i":
                                        if DEFER_POOL_FIN:
                                            deferred_fin.append((sumsl, sub))
                                        else:
                                            nc.gpsimd.tensor_tensor(
                                                out=sumsl, in0=sub[:, :, 0],
                                                in1=sub[:, :, 1], op=ADD,
                                            )
                                    else:
                                        nc.vector.tensor_tensor(
                                            out=sumsl, in0=sub[:, :, 0],
                                            in1=sub[:, :, 1], op=ADD,
                                        )
                                elif fin == 3:
                                    if f3t is None:
                                        f3t = tree_tp.tile([P, 2048], BF16, tag="f3")
                                    with nc.allow_low_precision(reason="bf16 tree"):
                                        nc.vector.tensor_tensor(
                                            out=f3t[:, :t], in0=sub[:, :, 0],
                                            in1=sub[:, :, 1], op=ADD,
                                        )
                                    nc.vector.tensor_tensor(
                                        out=sumsl, in0=f3t[:, :t],
                                        in1=sub[:, :, 2], op=ADD,
                                    )
                                else:
                                    nc.vector.tensor_reduce(
                                        out=sumsl, in_=sub, axis=X, op=ADD
                                    )
                        else:
                            for table, g0, t, soff in g["subs"]:
                                w = t * K
                                xo2 = (g["woff"] + soff * K * 2) // 2
                                sub = wt[:, xo2 : xo2 + -(-w // 2)].bitcast(BF16)[
                                    :, :w
                                ]
                                scur = sub.rearrange("p (t k) -> p t k", k=K)
                                scurw = K
                                tr = tree_tp.tile([P, WINDOW // 2], BF16, tag="tr")
                                with nc.allow_low_precision(reason="bf16 tree"):
                                    for hw in steps:
                                        dst = tr[:, : t * hw].rearrange(
                                            "p (t k) -> p t k", k=hw
                                        )
                                        nc.vector.tensor_tensor(
                                            out=dst,
                                            in0=scur[:, :, :hw],
                                            in1=scur[:, :, hw:scurw],
                                            op=ADD,
                                        )
                                        scur = dst
                                        scurw = hw
                                sumsl = bufs[table][:, g0 : g0 + t]
                                if fin == 2:
                                    nc.vector.tensor_tensor(
                                        out=sumsl,
                                        in0=scur[:, :, 0],
                                        in1=scur[:, :, 1],
                                        op=ADD,
                                    )
                                else:
                                    nc.vector.tensor_reduce(
                                        out=sumsl, in_=scur, axis=X, op=ADD
                                    )

                if "tail" in ABLATE:
                    zzt = node_tp.tile([P, 64], F32, tag="zz")
                    nc.vector.memset(zzt[:, :], 0.0)
                    nc.scalar.dma_start(out=out_d[:, :32], in_=zzt[:, :32])
                    continue
                for sumsl, sub in deferred_fin:
                    nc.gpsimd.tensor_tensor(
                        out=sumsl, in0=sub[:, :, 0], in1=sub[:, :, 1], op=ADD
                    )
                # elementwise tail, all on VectorE: den = isum+1 (in place),
                # agg = asum/den (hw divide), out = (agg+1) - x; split the
                # final op + output DMA in halves to overlap the drain
                DIV = mybir.AluOpType.divide
                ot = node_tp.tile([P, NR], F32, tag="ot")
                A = lambda tl: tl[:, :]
                if TAIL_ADD == "a":
                    nc.scalar.add(A(isum), A(isum), 1.0)
                else:
                    nc.vector.tensor_scalar_add(A(isum), A(isum), 1.0)
                rde = node_tp.tile([P, NR], F32, tag="rde")
                nc.vector.reciprocal_approx_fast(out=A(rde), in_=A(isum))
                nc.vector.tensor_tensor(out=A(ot), in0=A(asum), in1=A(rde), op=MUL)
                if ndg1:
                    NRp = NR + NR % 2
                    ob = node_tp.tile([P, NRp // 2], F32, tag="ob")
                    obb = ob[:, :].bitcast(BF16)
                    h = (NR // 4) * 2
                    with nc.allow_low_precision(reason="bf16 output"):
                        for lo, hi in ((0, h), (h, NR)):
                            nc.vector.scalar_tensor_tensor(
                                out=obb[:, lo:hi], in0=ot[:, lo:hi], scalar=1.0,
                                in1=xv[:, lo:hi], op0=ADD, op1=SUB,
                            )
                            outq = {"p": nc.gpsimd, "a": nc.scalar,
                                    "s": nc.sync}[OUT_Q]
                            outq.dma_start(
                                out=out_d[:, lo // 2 : -(-hi // 2)],
                                in_=ob[:, lo // 2 : -(-hi // 2)],
                            )
                else:
                    nc.vector.tensor_tensor(out=A(ot), in0=A(ot), in1=iv["nu"], op=MUL)
                    nc.vector.scalar_tensor_tensor(
                        out=A(rde), in0=iv["de"], scalar=-1.0, in1=A(xv),
                        op0=MUL, op1=MUL,
                    )
                    nc.vector.tensor_tensor(out=A(ot), in0=A(ot), in1=A(rde), op=ADD)
                    nc.vector.tensor_tensor(out=A(ot), in0=A(ot), in1=iv["gr"], op=ADD)
                    nc.scalar.dma_start(out=out_d[:, :], in_=ot[:, :])

    nc.finalize()
    return nc


def kernel(**inputs) -> np.ndarray:
    per_core, meta_orders, shapes = _pack(
        np.asarray(inputs["x"], np.float32),
        np.asarray(inputs["k_act"], np.float32),
        np.asarray(inputs["k_inh"], np.float32),
        np.asarray(inputs["nu"], np.float32),
        np.asarray(inputs["decay"], np.float32),
        np.asarray(inputs["growth"], np.float32),
        np.asarray(inputs["act_src"]),
        np.asarray(inputs["act_dst"]),
        np.asarray(inputs["inh_src"]),
        np.asarray(inputs["inh_dst"]),
    )
    nc = _build_nc(shapes)
    in_maps = [dict(per_core[c]) for c in range(N_CORES)]
    res = run_bass_kernel_spmd(nc, in_maps, list(range(N_CORES)))

    out_full = np.zeros(N_NODES, np.float32)
    nrows = shapes["nrows"]
    NR = shapes["NR"]
    for c in range(N_CORES):
        arr = res.results[c]["out"]
        if shapes["ndg1"]:
            # packed bf16 words -> f32 [P, NR]
            u = np.ascontiguousarray(arr).view(np.uint32)
            halves = np.empty((P, u.shape[1] * 2), np.uint16)
            halves[:, 0::2] = (u & 0xFFFF).astype(np.uint16)
            halves[:, 1::2] = (u >> 16).astype(np.uint16)
            arr = halves.view(ml_dtypes.bfloat16).astype(np.float32)[:, :NR]
        offN = 0
        for key, L in meta_orders[c]:
            nr = nrows[key]
            block = arr[:, offN : offN + nr].reshape(P * nr)
            valid = L >= 0
            out_full[L[valid]] = block[valid]
            offN += nr
    return out_full


# revision 26
# speedup vs baseline: 1.1008x; 1.1008x over previous
"""BioGNN message-passing kernel for 8 trn2 NeuronCores.

Strategy:
  - Shard by DESTINATION node range: core c owns nodes [c*125k, (c+1)*125k).
    Each edge is routed (host-side layout) to the core owning its dst, so no
    all-reduce is needed; the host concatenates per-core output slices.
  - Host does LAYOUT ONLY: per owned node, incoming edges are padded into
    dense ELL slabs binned by in-degree class; each slot carries bf16(x[src])
    (and bf16(k) when k is not all-ones). Nodes are ordered bin-major
    ((Ka,Ki) lexicographic) so per-class slab regions reduce into contiguous
    sum slices. Outputs are un-permuted on the host.
  - Device does ALL arithmetic: ScalarE (+ Pool for a slice share) squares
    the bf16 slots in place per window slice; VectorE reduces each K-group
    via a bf16 pairwise-halving tree (2x DVE mode) finishing in f32. Tree
    levels are SHARED across all same-K chunks (slab is laid out grouped by
    K), only the final level is per-destination-slice. Then the elementwise
    tail (num/den ratio, decay/growth terms) spread across engines.
  - num/den masks are folded into data/layout: asum rows for no-activator
    nodes are memset to 1 (isolated nodes to 0; bin (0,0) sorts first);
    nodes with act-degree 0 that were promoted into a padded class carry one
    pad slot of 1.0 so their reduced sum is exactly the mask value.
"""

import contextlib

import ml_dtypes
import numpy as np

import concourse.bacc as bacc
import concourse.mybir as mybir
import concourse.tile as tile
from concourse.bass_utils import run_bass_kernel_spmd

N_NODES = 1_000_000
N_CORES = 8
NPC = N_NODES // N_CORES
P = 128

WINDOW = 8192     # slab window width per partition, in bf16 words
TREE_MIN = 512    # groups narrower than this use per-sub tensor_reduce
RARE = 16384      # consolidate (Ka,Ki) pairs with fewer nodes than this
KCAP = 16         # rare pairs are promoted to at least this class
SQ_POOL_FRAC = 0.0  # Pool bf16 tensor ops are ~7x slower on HW than modeled
SQ_SLICE = 3072   # max bf16 words per square op (pipelining granularity)
POOL_FIN = False  # width-2 inh finals run on Pool instead of VectorE
ABLATE = frozenset()  # debug: subsets of {"dma","sq","tree","tail"} to skip
DEFER_POOL_FIN = True  # emit Pool finals after the window loop
TAIL_ADD = "a"    # engine for isum+=1: "a" ScalarE, "v" VectorE
OUT_Q = "s"       # queue issuing output DMAs: "a" ScalarE, "s" SP, "p" Pool

F32 = mybir.dt.float32
BF16 = mybir.dt.bfloat16


FINE_CLASSES = False

def _degree_classes(max_deg: int) -> list[int]:
    ks = (
        [4, 6, 8, 10, 12, 14, 16, 20, 24, 32]
        if FINE_CLASSES
        else [4, 6, 8, 12, 16, 24, 32]
    )
    ks = list(ks)
    while ks[-1] < max_deg:
        ks.append(ks[-1] * 2)
    return ks


def _class_of(deg: np.ndarray, ks: list[int]) -> np.ndarray:
    bounds = np.array(ks)
    idx = np.searchsorted(bounds, deg, side="left")
    out = np.zeros_like(deg)
    nz = deg > 0
    out[nz] = bounds[idx[nz]]
    return out


def _pack_bf16_words(arr):
    """[P, n] f32 -> [P, ceil(n/2)] f32 words holding round-to-nearest bf16."""
    a = arr.astype(ml_dtypes.bfloat16)
    if a.shape[1] % 2:
        a = np.concatenate([a, np.zeros((a.shape[0], 1), ml_dtypes.bfloat16)], axis=1)
    u = a.view(np.uint16)
    w = (u[:, 0::2].astype(np.uint32) | (u[:, 1::2].astype(np.uint32) << 16)).view(
        np.float32
    )
    return np.ascontiguousarray(w)


def _tree_steps(K: int):
    """Halving widths (bf16 tt levels) and the final width (f32 finish)."""
    w = K
    steps = []
    while w % 2 == 0 and w > 2:
        w //= 2
        steps.append(w)
    return steps, w


def _make_plan(all_keys, nrows, k1):
    """Group chunks by K class, pack class groups into windows.

    Returns (entries, windows, row_off, act_seg_rows, total_rows):
      entries: flat list of (table, K, g0, t, win, woff) for the packer,
        woff = bf16-word offset of the x region within the window (the k
        region, general path only, sits at woff + t*K).
      windows: list of {used, groups: [{K, woff, width, subs}]} where subs
        are (table, g0, t, soff) with soff the sub's slot offset (in slots
        of K words) within the group.
    """
    row_off = {}
    off = 0
    for key in all_keys:
        row_off[key] = off
        off += nrows[key]
    total_rows = off

    mult = 1 if k1 else 2
    # per-class chunk lists: act as one run of contiguous rows per class,
    # inh per bin
    act_seg_rows = {}
    by_k = {}
    for key in all_keys:
        Ka, Ki = key
        if Ka > 0 and Ka not in act_seg_rows:
            seg_rows = sum(nrows[k] for k in all_keys if k[0] == Ka)
            act_seg_rows[Ka] = (row_off[key], seg_rows)
            by_k.setdefault(Ka, []).append(("a", row_off[key], seg_rows))
        if Ki > 0:
            by_k.setdefault(Ki, []).append(("i", row_off[key], nrows[key]))

    # pack class groups into windows; split oversized groups at chunk
    # boundaries (chunks themselves split to fit WINDOW slots)
    entries = []
    windows = []
    cur = {"used": 0, "groups": []}

    def close_window():
        nonlocal cur
        if cur["groups"]:
            windows.append(cur)
            cur = {"used": 0, "groups": []}

    for K in sorted(by_k, key=lambda k: -sum(c[2] for c in by_k[k]) * k):
        max_slots = WINDOW // (K * mult)
        # split per-table runs into chunks of at most max_slots rows
        chunks = []
        for table, g0, rows in by_k[K]:
            r = 0
            while r < rows:
                t = min(max_slots, rows - r)
                chunks.append((table, g0 + r, t))
                r += t
        ci = 0
        while ci < len(chunks):
            free = WINDOW - cur["used"]
            grp_slots = free // (K * mult)
            if grp_slots < chunks[ci][2] and grp_slots < max_slots:
                close_window()
                continue
            grp = {"K": K, "woff": cur["used"], "subs": []}
            soff = 0
            while ci < len(chunks) and soff + chunks[ci][2] <= grp_slots:
                table, g0, t = chunks[ci]
                grp["subs"].append((table, g0, t, soff))
                entries.append((table, K, g0, t, len(windows),
                                grp["woff"] + soff * K * mult))
                soff += t
                ci += 1
            grp["width"] = soff * K
            cur["used"] += -(-(soff * K * mult) // 64) * 64
            cur["groups"].append(grp)
            if cur["used"] >= WINDOW - 64:
                close_window()
    close_window()
    return entries, windows, row_off, act_seg_rows, total_rows


def _pack(x, k_act, k_inh, nu, decay, growth, act_src, act_dst, inh_src, inh_dst):
    k1 = bool(np.all(k_act == 1.0) and np.all(k_inh == 1.0))
    ndg1 = bool(np.all(nu == 1.0) and np.all(decay == 1.0) and np.all(growth == 1.0))

    def sorted_table(src, dst, k):
        order = np.argsort(dst, kind="stable")
        deg = np.bincount(dst, minlength=N_NODES).astype(np.int64)
        rowptr = np.zeros(N_NODES + 1, np.int64)
        np.cumsum(deg, out=rowptr[1:])
        return src[order], k[order], deg, rowptr

    a_src, a_k, a_deg, a_ptr = sorted_table(act_src, act_dst, k_act)
    i_src, i_k, i_deg, i_ptr = sorted_table(inh_src, inh_dst, k_inh)

    max_deg = int(max(a_deg.max(), i_deg.max()))
    ks = _degree_classes(max_deg)
    nclasses = len(ks) + 1
    klist = [0] + ks

    ca = _class_of(a_deg, ks)
    ci = _class_of(i_deg, ks)

    # consolidate rare (ca, ci) pairs by cascading each into the cheapest
    # neighbour pair (bump one class up) until populous, so the device sees
    # few, large chunks without the padding blowup of a fixed promotion
    # target. Pair (0,0) (isolated nodes) is exempt: its rows must stay
    # identifiable so asum can be zeroed for them.
    karr = np.array([0] + ks)

    def up(c):
        i = int(np.searchsorted(karr, c)) + 1
        return int(karr[min(i, len(karr) - 1)])

    pair_id = ca * 1024 + ci
    uniq_p, cnt_p = np.unique(pair_id, return_counts=True)
    pop = {int(u): int(n) for u, n in zip(uniq_p, cnt_p)}
    remap = {}
    live = dict(pop)
    changed = True
    while changed:
        changed = False
        for pid in sorted(live, key=lambda q: live[q]):
            if live[pid] >= RARE or pid == 0:
                continue
            a, i = pid // 1024, pid % 1024
            cands = []
            ua, ui = up(a), up(i)
            if ua != a:
                cands.append(((ua - a), ua * 1024 + i))
            if ui != i:
                cands.append(((ui - i), a * 1024 + ui))
            if not cands:
                continue
            cands.sort()
            _, tgt = cands[0]
            remap[pid] = tgt
            live[tgt] = live.get(tgt, 0) + live[pid]
            del live[pid]
            changed = True
            break

    def resolve(pid):
        while pid in remap:
            pid = remap[pid]
        return pid

    if remap:
        res = {int(u): resolve(int(u)) for u in uniq_p}
        new_id = np.vectorize(res.get, otypes=[np.int64])(pair_id)
        ca = (new_id // 1024).astype(ca.dtype)
        ci = (new_id % 1024).astype(ci.dtype)

    x32 = x.astype(np.float32)
    xbf = x32.astype(ml_dtypes.bfloat16).astype(np.float32)

    core_bins = []
    for c in range(N_CORES):
        lo, hi = c * NPC, (c + 1) * NPC
        nodes = np.arange(lo, hi)
        binid = np.searchsorted(np.array(klist), ca[lo:hi]) * nclasses + np.searchsorted(
            np.array(klist), ci[lo:hi]
        )
        order = np.argsort(binid, kind="stable")
        nodes_sorted = nodes[order]
        binid_sorted = binid[order]
        uniq, starts = np.unique(binid_sorted, return_index=True)
        ends = np.append(starts[1:], len(binid_sorted))
        bins = {}
        for u, s, e in zip(uniq, starts, ends):
            bins[(klist[u // nclasses], klist[u % nclasses])] = nodes_sorted[s:e]
        core_bins.append(bins)

    all_keys = sorted({k for b in core_bins for k in b.keys()})
    nrows = {}
    for key in all_keys:
        nmax = max(len(b.get(key, ())) for b in core_bins)
        nrows[key] = -(-nmax // P)

    entries, windows, row_off, act_seg_rows, total_rows = _make_plan(
        all_keys, nrows, k1
    )

    r000 = nrows.get((0, 0), 0) if all_keys and all_keys[0] == (0, 0) else 0
    rka0 = sum(nrows[k] for k in all_keys if k[0] == 0)
    iz_runs = [(row_off[k], nrows[k]) for k in all_keys if k[1] == 0]

    def build_slab(L, K, rowptr, deg, srcs, kvals, act_pad1):
        Lc = L.clip(0)
        d = np.where(L >= 0, deg[Lc], 0)
        base = rowptr[Lc]
        cols = np.arange(K)
        idx2 = base[:, None] + cols[None, :]
        valid = cols[None, :] < d[:, None]
        idxc = np.where(valid, idx2, 0)
        sx = np.where(valid, xbf[srcs[idxc]], np.float32(0))
        sk = np.where(valid, kvals[idxc], np.float32(0)) if not k1 else None
        if act_pad1:
            z = (L >= 0) & (d == 0)
            if z.any():
                sx[z, 0] = 1.0
                if sk is not None:
                    sk[z, 0] = 1.0
        return sx, sk

    win_used = [w0["used"] for w0 in windows]
    win_start = [0]
    for u in win_used[:-1]:
        win_start.append(win_start[-1] + u)

    per_core = []
    meta_orders = []
    for c in range(N_CORES):
        bins = core_bins[c]
        ax_parts = {}
        ak_parts = {}
        ix_bin = {}
        ik_bin = {}
        xv_l = []
        ndg_l = {"nu": [], "de": [], "gr": []}
        orders = []
        for key in all_keys:
            Ka, Ki = key
            nr = nrows[key]
            L = np.full(nr * P, -1, np.int64)
            have = bins.get(key)
            if have is not None:
                L[: len(have)] = have
            orders.append((key, L))
            if Ka > 0:
                sx, sk = build_slab(L, Ka, a_ptr, a_deg, a_src, a_k, True)
                ax_parts.setdefault(Ka, []).append(sx.reshape(P, nr * Ka))
                if sk is not None:
                    ak_parts.setdefault(Ka, []).append(sk.reshape(P, nr * Ka))
            if Ki > 0:
                sx, sk = build_slab(L, Ki, i_ptr, i_deg, i_src, i_k, False)
                ix_bin[key] = sx.reshape(P, nr * Ki)
                if sk is not None:
                    ik_bin[key] = sk.reshape(P, nr * Ki)
            valid = L >= 0
            Lc = L.clip(0)

            def pk(v):
                return (
                    np.where(valid, v[Lc], np.float32(0))
                    .astype(np.float32)
                    .reshape(P, nr)
                )

            xv_l.append(pk(x32))
            if not ndg1:
                ndg_l["nu"].append(pk(nu))
                ndg_l["de"].append(pk(decay))
                ndg_l["gr"].append(pk(growth))

        ax_seg = {Ka: np.concatenate(p, axis=1) for Ka, p in ax_parts.items()}
        ak_seg = {Ka: np.concatenate(p, axis=1) for Ka, p in ak_parts.items()}

        slab = np.zeros((P, (win_start[-1] + win_used[-1]) // 2), np.float32)
        for table, K, g0, t, win, woff in entries:
            w = t * K
            base = win_start[win] + woff
            if table == "a":
                seg0, _ = act_seg_rows[K]
                r0 = g0 - seg0
                sx = ax_seg[K][:, r0 * K : (r0 + t) * K]
                sk = ak_seg[K][:, r0 * K : (r0 + t) * K] if not k1 else None
            else:
                key = next(
                    kk for kk in all_keys
                    if kk[1] == K and row_off[kk] <= g0 < row_off[kk] + nrows[kk]
                )
                r0 = g0 - row_off[key]
                sx = ix_bin[key][:, r0 * K : (r0 + t) * K]
                sk = ik_bin[key][:, r0 * K : (r0 + t) * K] if not k1 else None
            slab[:, base // 2 : base // 2 + w // 2] = _pack_bf16_words(sx)
            if sk is not None:
                kb = base + w
                slab[:, kb // 2 : kb // 2 + w // 2] = _pack_bf16_words(sk)

        core = {"slab": slab, "nodevf": np.ascontiguousarray(np.concatenate(xv_l, axis=1))}
        if not ndg1:
            ndg = np.concatenate(
                [np.concatenate(ndg_l[nm], axis=1) for nm in ("nu", "de", "gr")],
                axis=1,
            )
            core["nodevb"] = _pack_bf16_words(ndg)
        per_core.append(core)
        meta_orders.append(orders)

    shapes = {
        "keys": all_keys,
        "nrows": nrows,
        "NR": total_rows,
        "entries": entries,
        "windows": windows,
        "win_used": win_used,
        "win_start": win_start,
        "r000": r000,
        "rka0": rka0,
        "iz_runs": iz_runs,
        "k1": k1,
        "ndg1": ndg1,
    }
    assert per_core[0]["nodevf"].shape[1] == shapes["NR"]
    return per_core, meta_orders, shapes


def _build_nc(shapes, loop_R=None, unroll=1):
    NR = shapes["NR"]
    windows = shapes["windows"]
    win_used = shapes["win_used"]
    win_start = shapes["win_start"]
    r000 = shapes["r000"]
    rka0 = shapes["rka0"]
    iz_runs = shapes["iz_runs"]
    k1 = shapes["k1"]
    ndg1 = shapes["ndg1"]

    nc = bacc.Bacc(None, target_bir_lowering=False)
    WT2 = (win_start[-1] + win_used[-1]) // 2
    sl_d = nc.declare_dram_parameter("slab", [P, WT2], F32, isOutput=False)
    nvf_d = nc.declare_dram_parameter("nodevf", [P, NR], F32, isOutput=False)
    if not ndg1:
        NB = -(-3 * NR // 2)
        nvb_d = nc.declare_dram_parameter("nodevb", [P, NB], F32, isOutput=False)
    out_d = nc.declare_dram_parameter("out", [P, NR], F32, isOutput=True)

    MUL = mybir.AluOpType.mult
    ADD = mybir.AluOpType.add
    SUB = mybir.AluOpType.subtract
    X = mybir.AxisListType.X

    # square-slice engine assignment: greedy fill Pool up to SQ_POOL_FRAC
    total_sq = sum(g["width"] for w0 in windows for g in w0["groups"])
    pool_budget = total_sq * SQ_POOL_FRAC

    with tile.TileContext(nc) as tc:
        with (
            tc.tile_pool(name="slab", bufs=4) as slab_tp,
            tc.tile_pool(name="tree", bufs=3) as tree_tp,
            tc.tile_pool(name="sums", bufs=1) as sums_tp,
            tc.tile_pool(name="node", bufs=1) as node_tp,
        ):
            U = unroll
            if loop_R:
                for cand in (8, 4, 2, 1):
                    if loop_R % cand == 0:
                        U = cand
                        break
            loop_cm = (
                tc.For_i(0, loop_R // U, 1) if loop_R else contextlib.nullcontext()
            )
            with loop_cm:
              for _body_rep in range(U):
                asum = sums_tp.tile([P, NR], F32, tag="asum")
                isum = sums_tp.tile([P, NR], F32, tag="isum")
                if r000 > 0:
                    nc.vector.memset(asum[:, :r000], 0.0)
                if rka0 > r000:
                    nc.vector.memset(asum[:, r000:rka0], 1.0)
                for z0, zn in iz_runs:
                    nc.vector.memset(isum[:, z0 : z0 + zn], 0.0)

                xv = node_tp.tile([P, NR], F32, tag="xv")
                nc.scalar.dma_start(out=xv[:, :], in_=nvf_d[:, :])
                if not ndg1:
                    NB = -(-3 * NR // 2)
                    nvb = node_tp.tile([P, NB], F32, tag="nvb")
                    nc.scalar.dma_start(out=nvb[:, :], in_=nvb_d[:, :])
                    nvb_b = nvb[:, :].bitcast(BF16)
                    iv = {
                        nm: nvb_b[:, j * NR : (j + 1) * NR]
                        for j, nm in enumerate(("nu", "de", "gr"))
                    }

                sq_state = {"a": 0.0, "p": 0.0}

                def square_region(xs, width):
                    # slice into <=SQ_SLICE pieces, balance ACT vs Pool by
                    # weighted time (ACT 0.833 ns/el, Pool 1.98 ns/el)
                    o = 0
                    while o < width:
                        w = min(SQ_SLICE, width - o)
                        sl = xs[:, o : o + w]
                        use_pool = (
                            SQ_POOL_FRAC > 0
                            and sq_state["p"] + w
                            <= (sq_state["a"] + sq_state["p"] + w) * SQ_POOL_FRAC
                        )
                        if use_pool:
                            with nc.allow_low_precision(reason="bf16 squares"):
                                nc.gpsimd.tensor_tensor(out=sl, in0=sl, in1=sl, op=MUL)
                            sq_state["p"] += w
                        else:
                            nc.scalar.square(out=sl, in_=sl)
                            sq_state["a"] += w
                        o += w

                bufs = {"a": asum, "i": isum}
                deferred_fin = []
                for win, w0 in enumerate(windows):
                    used2 = -(-win_used[win] // 2)
                    wt = slab_tp.tile([P, WINDOW // 2], F32, tag="win")
                    ws2 = win_start[win] // 2
                    # split the window DMA at a group boundary near the middle
                    split2 = 0
                    for g in w0["groups"]:
                        if g["woff"] // 2 >= used2 // 2:
                            split2 = g["woff"] // 2
                            break
                    if "dma" not in ABLATE:
                        if 0 < split2 < used2:
                            nc.sync.dma_start(
                                out=wt[:, :split2], in_=sl_d[:, ws2 : ws2 + split2]
                            )
                            nc.sync.dma_start(
                                out=wt[:, split2:used2],
                                in_=sl_d[:, ws2 + split2 : ws2 + used2],
                            )
                        else:
                            nc.sync.dma_start(
                                out=wt[:, :used2], in_=sl_d[:, ws2 : ws2 + used2]
                            )
                    else:
                        nc.vector.memset(wt[:, :64], 1.0)
                    for g in w0["groups"]:
                        K = g["K"]
                        width = g["width"]
                        if width == 0:
                            continue
                        o2 = g["woff"] // 2
                        xs = wt[:, o2 : o2 + -(-width // 2)].bitcast(BF16)[:, :width]
                        if k1:
                            if "sq" not in ABLATE:
                                square_region(xs, width)
                        else:
                            # per-sub k regions sit interleaved: sub x at
                            # soff*K*2, k at soff*K*2 + t*K (see _make_plan)
                            with nc.allow_low_precision(reason="bf16 edge gains"):
                                for table, g0, t, soff in g["subs"]:
                                    w = t * K
                                    xo2 = (g["woff"] + soff * K * 2) // 2
                                    xsk = wt[:, xo2 : xo2 + -(-w // 2)].bitcast(
                                        BF16
                                    )[:, :w]
                                    kS = wt[
                                        :, xo2 + -(-w // 2) : xo2 + 2 * -(-w // 2)
                                    ].bitcast(BF16)[:, :w]
                                    square_region(xsk, w)
                                    nc.vector.tensor_tensor(
                                        out=xsk, in0=xsk, in1=kS, op=MUL
                                    )
                        steps, fin = _tree_steps(K)
                        if "tree" in ABLATE:
                            continue
                        if width < TREE_MIN:
                            for table, g0, t, soff in g["subs"]:
                                if k1:
                                    sub = xs[:, soff * K : (soff + t) * K]
                                else:
                                    xo2 = (g["woff"] + soff * K * 2) // 2
                                    sub = wt[:, xo2 : xo2 + -(-(t * K) // 2)].bitcast(
                                        BF16
                                    )[:, : t * K]
                                nc.vector.tensor_reduce(
                                    out=bufs[table][:, g0 : g0 + t],
                                    in_=sub.rearrange("p (t k) -> p t k", k=K),
                                    axis=X,
                                    op=ADD,
                                )
                            continue
                        # shared halving levels over the whole group
                        if k1:
                            cur = xs.rearrange("p (t k) -> p t k", k=K)
                        else:
                            # general path: x slots per sub are strided by
                            # the k region; fall back to per-sub trees
                            cur = None
                        curw = K
                        if cur is not None:
                            tr = tree_tp.tile([P, WINDOW // 2], BF16, tag="tr")
                            tslots = width // K
                            with nc.allow_low_precision(reason="bf16 tree"):
                                for hw in steps:
                                    dst = tr[:, : tslots * hw].rearrange(
                                        "p (t k) -> p t k", k=hw
                                    )
                                    nc.vector.tensor_tensor(
                                        out=dst,
                                        in0=cur[:, :, :hw],
                                        in1=cur[:, :, hw:curw],
                                        op=ADD,
                                    )
                                    cur = dst
                                    curw = hw
                            f3t = None
                            for table, g0, t, soff in g["subs"]:
                                sub = cur[:, soff : soff + t, :]
                                sumsl = bufs[table][:, g0 : g0 + t]
                                if fin == 2:
                                    if POOL_FIN and table == "i":
                                        if DEFER_POOL_FIN:
                                            deferred_fin.append((sumsl, sub))
                                        else:
                                            nc.gpsimd.tensor_tensor(
                                                out=sumsl, in0=sub[:, :, 0],
                                                in1=sub[:, :, 1], op=ADD,
                                            )
                                    else:
                                        nc.vector.tensor_tensor(
                                            out=sumsl, in0=sub[:, :, 0],
                                            in1=sub[:, :, 1], op=ADD,
                                        )
                                elif fin == 3:
                                    if f3t is None:
                                        f3t = tree_tp.tile([P, 2048], BF16, tag="f3")
                                    with nc.allow_low_precision(reason="bf16 tree"):
                                        nc.vector.tensor_tensor(
                                            out=f3t[:, :t], in0=sub[:, :, 0],
                                            in1=sub[:, :, 1], op=ADD,
                                        )
                                    nc.vector.tensor_tensor(
                                        out=sumsl, in0=f3t[:, :t],
                                        in1=sub[:, :, 2], op=ADD,
                                    )
                                else:
                                    nc.vector.tensor_reduce(
                                        out=sumsl, in_=sub, axis=X, op=ADD
                                    )
                        else:
                            for table, g0, t, soff in g["subs"]:
                                w = t * K
                                xo2 = (g["woff"] + soff * K * 2) // 2
                                sub = wt[:, xo2 : xo2 + -(-w // 2)].bitcast(BF16)[
                                    :, :w
                                ]
                                scur = sub.rearrange("p (t k) -> p t k", k=K)
                                scurw = K
                                tr = tree_tp.tile([P, WINDOW // 2], BF16, tag="tr")
                                with nc.allow_low_precision(reason="bf16 tree"):
                                    for hw in steps:
                                        dst = tr[:, : t * hw].rearrange(
                                            "p (t k) -> p t k", k=hw
                                        )
                                        nc.vector.tensor_tensor(
                                            out=dst,
                                            in0=scur[:, :, :hw],
                                            in1=scur[:, :, hw:scurw],
                                            op=ADD,
                                        )
                                        scur = dst
                                        scurw = hw
                                sumsl = bufs[table][:, g0 : g0 + t]
                                if fin == 2:
                                    nc.vector.tensor_tensor(
                                        out=sumsl,
                                        in0=scur[:, :, 0],
                                        in1=scur[:, :, 1],
                                        op=ADD,
                                    )
                                else:
                                    nc.vector.tensor_reduce(
                                        out=sumsl, in_=scur, axis=X, op=ADD
                                    )

                if "tail" in ABLATE:
                    zzt = node_tp.tile([P, 64], F32, tag="zz")
                    nc.vector.memset(zzt[:, :], 0.0)
                    nc.scalar.dma_start(out=out_d[:, :32], in_=zzt[:, :32])
                    continue
                for sumsl, sub in deferred_fin:
                    nc.gpsimd.tensor_tensor(
                        out=sumsl, in0=sub[:, :, 0], in1=sub[:, :, 1], op=ADD
                    )
                # elementwise tail, all on VectorE: den = isum+1 (in place),
                # agg = asum/den (hw divide), out = (agg+1) - x; split the
                # final op + output DMA in halves to overlap the drain
                DIV = mybir.AluOpType.divide
                ot = node_tp.tile([P, NR], F32, tag="ot")
                A = lambda tl: tl[:, :]
                if TAIL_ADD == "a":
                    nc.scalar.add(A(isum), A(isum), 1.0)
                else:
                    nc.vector.tensor_scalar_add(A(isum), A(isum), 1.0)
                rde = node_tp.tile([P, NR], F32, tag="rde")
                nc.vector.reciprocal_approx_fast(out=A(rde), in_=A(isum))
                nc.vector.tensor_tensor(out=A(ot), in0=A(asum), in1=A(rde), op=MUL)
                if ndg1:
                    NRp = NR + NR % 2
                    ob = node_tp.tile([P, NRp // 2], F32, tag="ob")
                    obb = ob[:, :].bitcast(BF16)
                    h = (NR // 4) * 2
                    with nc.allow_low_precision(reason="bf16 output"):
                        for lo, hi in ((0, h), (h, NR)):
                            nc.vector.scalar_tensor_tensor(
                                out=obb[:, lo:hi], in0=ot[:, lo:hi], scalar=1.0,
                                in1=xv[:, lo:hi], op0=ADD, op1=SUB,
                            )
                            outq = {"p": nc.gpsimd, "a": nc.scalar,
                                    "s": nc.sync}[OUT_Q]
                            outq.dma_start(
                                out=out_d[:, lo // 2 : -(-hi // 2)],
                                in_=ob[:, lo // 2 : -(-hi // 2)],
                            )
                else:
                    nc.vector.tensor_tensor(out=A(ot), in0=A(ot), in1=iv["nu"], op=MUL)
                    nc.vector.scalar_tensor_tensor(
                        out=A(rde), in0=iv["de"], scalar=-1.0, in1=A(xv),
                        op0=MUL, op1=MUL,
                    )
                    nc.vector.tensor_tensor(out=A(ot), in0=A(ot), in1=A(rde), op=ADD)
                    nc.vector.tensor_tensor(out=A(ot), in0=A(ot), in1=iv["gr"], op=ADD)
                    nc.scalar.dma_start(out=out_d[:, :], in_=ot[:, :])

    nc.finalize()
    return nc


def kernel(**inputs) -> np.ndarray:
    per_core, meta_orders, shapes = _pack(
        np.asarray(inputs["x"], np.float32),
        np.asarray(inputs["k_act"], np.float32),
        np.asarray(inputs["k_inh"], np.float32),
        np.asarray(inputs["nu"], np.float32),
        np.asarray(inputs["decay"], np.float32),
        np.asarray(inputs["growth"], np.float32),
        np.asarray(inputs["act_src"]),
        np.asarray(inputs["act_dst"]),
        np.asarray(inputs["inh_src"]),
        np.asarray(inputs["inh_dst"]),
    )
    nc = _build_nc(shapes)
    in_maps = [dict(per_core[c]) for c in range(N_CORES)]
    res = run_bass_kernel_spmd(nc, in_maps, list(range(N_CORES)))

    out_full = np.zeros(N_NODES, np.float32)
    nrows = shapes["nrows"]
    NR = shapes["NR"]
    for c in range(N_CORES):
        arr = res.results[c]["out"]
        if shapes["ndg1"]:
            # packed bf16 words -> f32 [P, NR]
            u = np.ascontiguousarray(arr).view(np.uint32)
            halves = np.empty((P, u.shape[1] * 2), np.uint16)
            halves[:, 0::2] = (u & 0xFFFF).astype(np.uint16)
            halves[:, 1::2] = (u >> 16).astype(np.uint16)
            arr = halves.view(ml_dtypes.bfloat16).astype(np.float32)[:, :NR]
        offN = 0
        for key, L in meta_orders[c]:
            nr = nrows[key]
            block = arr[:, offN : offN + nr].reshape(P * nr)
            valid = L >= 0
            out_full[L[valid]] = block[valid]
            offN += nr
    return out_full
